# revision 7
# baseline (speedup 1.0000x reference)
"""BrightnessLoss Trainium2 kernel (raw Bass, 8-core data parallel), v5.

reference:
    V(x)   = max_c(clip(x, 0, 1))        over channel dim (RGB)
    result = mean(|V(pred) - V(target)|) over (N, H, W)

Identities (bulk units, ACT-assisted path):
    u := relu(max3)  (free in the DVE stt: (m1 max 0) max B)
    W := Relu(1 - u) == 1 - clip(max3, 0, 1)
    |Vp - Vt| == |Wp - Wt|;  sum|Wp-Wt| == 2*sum max(Wp,Wt) - sum Wp - sum Wt
Last unit (DVE-only, no ACT round trip):
    vp := min(relu(max3_p), 1) == V(pred)
    sum|vp - vt| == sum max(vp,vt) - sum min(vp,vt)

Schedule notes (HBM-bound at ~358 GB/s/core):
  - Arrivals must TAPER: a full 1024 unit landing near stream-end leaves
    ~7us of serialized DVE work after the last byte.  11 units: imgs 0-2
    in 1024 halves; img 3 split (512A, 512S, 256S, 512A, 256A) so the
    last four arrivals are small and spaced ~2.2us apart.
  - Rings: SP (sync) u0,u2,u4,u7,u8 (11.8 MB); ACT (scalar)
    u1,u3,u5,u6,u9,u10 (13.4 MB).  SP drains ~4us early; ACT streams the
    tiny tail alone at full rate, ending with the DVE-only 256-wide u10.
  - Units never narrower than 256 (1KB per-partition descriptors); the
    512B-desc regime streams at a fraction of line rate.
  - 4-deep input buffers: every DMA issue waits only on DVE stts four
    units back — rings never starve, sequencers never hit ring credits
    (max ~2 queued per ring).
  - ONE output DMA at the very end from the long-quiet SP ring; receipt
    hides under the fixed end-of-NEFF semaphore sweep.
  - Bass.__init__'s all-engine barrier is suppressed (earlier first DMA);
    a gpsimd guard sem orders the const-1.0 bias tile, and a warm-up Relu
    pulls the lazy ACT_TABLE_LOAD off the W_0 critical path.
Host combines partials in float64.
"""

import numpy as np

N_CORES = 8
N_IMG = 4  # 32 / 8
C = 3
P = 128
F = 2048  # 512*512 / 128
N_PIX = 32 * 512 * 512

# (img, col_offset, width, ring)  ring: 0 = SP/sync, 1 = ACT/scalar
UNITS = [
    (0, 0, 1024, 0),
    (0, 1024, 1024, 1),
    (1, 0, 1024, 0),
    (1, 1024, 1024, 1),
    (2, 0, 1024, 0),
    (2, 1024, 1024, 1),
    (3, 0, 512, 1),
    (3, 512, 512, 0),
    (3, 1024, 256, 0),
    (3, 1280, 512, 1),
    (3, 1792, 256, 1),  # last: DVE-only, lands last on the ACT ring
]
N_UNITS = len(UNITS)
N_BULK = N_UNITS - 1  # units 0..9 use the ACT W path
SLOTS = 4
W_LAST = UNITS[-1][2]
N_COLS = 3 * N_BULK + 2  # 30 bulk cols + (sum max, sum min) for u10


def _build_program():
    from contextlib import ExitStack

    import concourse.bass as bass
    import concourse.mybir as mybir

    fp32 = mybir.dt.float32
    Alu = mybir.AluOpType
    Act = mybir.ActivationFunctionType

    # Suppress the framework barrier at the end of Bass.__init__ (after the
    # const-AP memsets): engines enter the body without a rendezvous and the
    # first input DMA issues earlier.  The only preamble state the body reads
    # is the const-1.0 bias tile (ACT Relu bias); the gpsimd guard sem below
    # re-establishes that one ordering edge.
    _cls_aeb = bass.Bass.all_engine_barrier
    bass.Bass.all_engine_barrier = lambda *a, **k: None
    try:
        # detect_race_conditions=False: the raw-mode CoreSim race detector
        # can't see same-engine program-order (DVE TT -> STT RAW); hardware
        # engines execute in order.
        nc = bass.Bass(
            "TRN2",
            target_bir_lowering=False,
            debug=False,
            detect_race_conditions=False,
        )
    finally:
        bass.Bass.all_engine_barrier = _cls_aeb

    pred = nc.dram_tensor("pred", [N_IMG, C, P, F], fp32, kind="ExternalInput").ap()
    targ = nc.dram_tensor("target", [N_IMG, C, P, F], fp32, kind="ExternalInput").ap()
    out = nc.dram_tensor("partials", [P, N_COLS], fp32, kind="ExternalOutput").ap()

    fc = 1024  # max unit width

    with ExitStack() as ctx:
        sb = lambda name, shape: ctx.enter_context(nc.sbuf_tensor(name, shape, fp32))
        sem = lambda name: ctx.enter_context(nc.semaphore(name))

        inb = [[sb(f"in{sl}{s}", [P, C * fc]) for s in range(2)] for sl in range(SLOTS)]
        ub = [[sb(f"u{sl}{s}", [P, fc]) for s in range(2)] for sl in range(2)]
        wb = [[sb(f"w{sl}{s}", [P, fc]) for s in range(2)] for sl in range(2)]
        m1 = sb("m1", [P, fc])
        scr = sb("scratch", [P, fc])
        rawp = sb("rawp", [P, W_LAST])
        vp = sb("vp", [P, W_LAST])
        acc = sb("acc", [P, N_COLS])
        guard_buf = sb("guard_buf", [P, 1])
        warm = sb("act_warm", [P, 1])

        ip = [sem(f"ip{s}") for s in range(SLOTS)]  # pred DMA done, per slot
        it = [sem(f"it{s}") for s in range(SLOTS)]  # targ DMA done, per slot
        u_sem = sem("u")      # DVE stt per unit-side (2/unit)
        act_sem = sem("act")  # ACT W per unit-side (2/unit)
        gp_sem = sem("gp")    # DVE accums (10 bulk + 2 last = 12)
        out_sem = sem("outd")
        cready = sem("cready")  # gpsimd: const-AP memsets retired

        slot_of = [u % SLOTS for u in range(N_UNITS)]
        use_of = [u // SLOTS for u in range(N_UNITS)]

        def dma_in(eng, side_idx, u):
            img, off, w, _ = UNITS[u]
            side = (pred, targ)[side_idx]
            s_sem = (ip, it)[side_idx][slot_of[u]]
            src = side[img, :, :, off : off + w].rearrange("c p f -> p c f")
            eng.dma_start(
                out=inb[slot_of[u]][side_idx][:, : C * w].rearrange(
                    "p (c f) -> p c f", c=C
                ),
                in_=src,
            ).then_inc(s_sem, 16)

        def war_waits(eng, u):
            # WAR on inb[u%4]: stts of unit u-4 (its last reader) must be done
            # before the pred/targ DMA of unit u may overwrite the slot.
            if u >= SLOTS:
                eng.wait_ge(u_sem, 2 * (u - SLOTS) + 1)
            yield
            if u >= SLOTS:
                eng.wait_ge(u_sem, 2 * (u - SLOTS) + 2)
            yield

        def paced_dma(eng, u):
            g = war_waits(eng, u)
            next(g)
            dma_in(eng, 0, u)
            next(g)
            dma_in(eng, 1, u)

        block = ctx.enter_context(nc.Block(no_gpsimd_drain=True))

        @block.gpsimd
        def _(gp):
            # Runs after the preamble const-AP memsets in gpsimd program
            # order; signals ACT that the const-1.0 bias tile is valid.
            gp.memset(guard_buf[:, :], 0.0).then_inc(cready, 1)

        @block.sync
        def _(sync):
            # SP ring: u0, u2 up front, then WAR-paced; single output DMA at
            # the very end (ring quiet since ~T-4.4us, receipts retired).
            for u in (0, 2, 4, 7, 8):
                paced_dma(sync, u)
            sync.wait_ge(gp_sem, N_BULK + 2)
            # No out_sem wait after the write: the block-exit drain fences
            # the ring; the receipt hides under the end-of-NEFF sem sweep.
            sync.dma_start(out=out[:, :], in_=acc[:, :]).then_inc(out_sem, 16)

        @block.vector
        def _(vector):
            def accum(u):
                # max(Wp, Wt) elementwise, accum_out = per-partition sum
                w = UNITS[u][2]
                vector.wait_ge(act_sem, 2 * (u + 1))
                vector.scalar_tensor_tensor(
                    scr[:, :w],
                    wb[u % 2][0][:, :w],
                    0.0,
                    wb[u % 2][1][:, :w],
                    op0=Alu.bypass,
                    op1=Alu.max,
                    accum_out=acc[:, 3 * u : 3 * u + 1],
                ).then_inc(gp_sem, 1)

            for u in range(N_BULK):
                w = UNITS[u][2]
                sl, k = slot_of[u], use_of[u]
                t = inb[sl]
                for s in range(2):
                    vector.wait_ge((ip, it)[s][sl], 16 * (k + 1))
                    vector.tensor_max(m1[:, :w], t[s][:, 0:w], t[s][:, w : 2 * w])
                    if u >= 2:
                        # WAR on ub[u%2][s]: ACT's W of unit u-2 (its reader)
                        vector.wait_ge(act_sem, 2 * (u - 1))
                    vector.scalar_tensor_tensor(
                        ub[u % 2][s][:, :w],
                        m1[:, :w],
                        0.0,
                        t[s][:, 2 * w : 3 * w],
                        op0=Alu.max,
                        op1=Alu.max,
                    ).then_inc(u_sem, 1)
                if u > 0:
                    accum(u - 1)

            # u10: DVE-only, w=256, lands last on the ACT ring.  Pred-side
            # ops overlap the targ half's stream; accum(9) fills the gap
            # while ACT finishes W_9.
            u = N_UNITS - 1
            w = UNITS[u][2]
            sl, k = slot_of[u], use_of[u]
            t = inb[sl]
            vector.wait_ge(ip[sl], 16 * (k + 1))
            vector.tensor_max(m1[:, :w], t[0][:, 0:w], t[0][:, w : 2 * w])
            # rawp = relu(max3(pred)) = (m1 max 0) max B
            vector.scalar_tensor_tensor(
                rawp[:, :w], m1[:, :w], 0.0, t[0][:, 2 * w : 3 * w],
                op0=Alu.max, op1=Alu.max,
            )
            # vp = clip(max3_p, 0, 1) = (rawp min 1) min rawp  (min idempotent)
            vector.scalar_tensor_tensor(
                vp[:, :w], rawp[:, :w], 1.0, rawp[:, :w],
                op0=Alu.min, op1=Alu.min,
            )
            accum(N_BULK - 1)
            vector.wait_ge(it[sl], 16 * (k + 1))
            vector.tensor_max(m1[:, :w], t[1][:, 0:w], t[1][:, w : 2 * w])
            vector.scalar_tensor_tensor(
                rawp[:, :w], m1[:, :w], 0.0, t[1][:, 2 * w : 3 * w],
                op0=Alu.max, op1=Alu.max,
            )
            # sum max(vp,vt), sum min(vp,vt);  vt = (rawt min 1)
            vector.scalar_tensor_tensor(
                scr[:, :w], rawp[:, :w], 1.0, vp[:, :w],
                op0=Alu.min, op1=Alu.max,
                accum_out=acc[:, 3 * N_BULK : 3 * N_BULK + 1],
            ).then_inc(gp_sem, 1)
            vector.scalar_tensor_tensor(
                scr[:, :w], rawp[:, :w], 1.0, vp[:, :w],
                op0=Alu.min, op1=Alu.min,
                accum_out=acc[:, 3 * N_BULK + 1 : 3 * N_BULK + 2],
            ).then_inc(gp_sem, 1)

        @block.scalar
        def _(scalar):
            # ACT ring: u1, u3 up front; later units WAR-paced via u_sem
            # (never behind a W compute).  The warm-up Relu pulls the lazy
            # ACT_TABLE_LOAD off the W_0 critical path.
            dma_in(scalar, 0, 1)
            dma_in(scalar, 1, 1)
            dma_in(scalar, 0, 3)
            dma_in(scalar, 1, 3)
            scalar.wait_ge(cready, 1)  # const-1.0 bias tile valid
            scalar.activation(warm[:, :], guard_buf[:, :], Act.Relu, bias=1.0)

            def W(n):
                w = UNITS[n][2]
                for s in range(2):
                    scalar.wait_ge(u_sem, 2 * n + s + 1)
                    if n >= 2:
                        # WAR on wb[n%2][s]: accum of unit n-2 (its reader)
                        scalar.wait_ge(gp_sem, n - 1)
                    scalar.activation(
                        wb[n % 2][s][:, :w],
                        ub[n % 2][s][:, :w],
                        Act.Relu,
                        bias=1.0,
                        scale=-1.0,
                        accum_out=acc[:, 3 * n + 1 + s : 3 * n + 2 + s],
                    ).then_inc(act_sem, 1)

            W(0)
            paced_dma(scalar, 5)
            W(1)
            paced_dma(scalar, 6)
            W(2)
            W(3)
            W(4)
            paced_dma(scalar, 9)
            W(5)
            paced_dma(scalar, 10)
            W(6)
            W(7)
            W(8)
            W(9)

        # Skip the Block-exit all-engine barrier: every cross-engine
        # dependency is semaphore-gated and the per-engine exit drains
        # (no_gpsimd_drain path) still fence the DMA rings, so engines may
        # halt independently — NEFF completion waits for all engines anyway.
        nc.all_engine_barrier = lambda *a, **k: None

    del nc.all_engine_barrier  # restore class method
    return nc


_program = None


def _get_program():
    global _program
    if _program is None:
        _program = _build_program()
    return _program


def _finish(partials_list):
    """partials_list: per-core [P, N_COLS] f32.
    Bulk unit u cols [3u, 3u+1, 3u+2] = [sum max(Wp,Wt), sum Wp, sum Wt]:
      sum|Vp-Vt| over the unit = 2*col0 - col1 - col2.
    Last unit cols [30, 31] = [sum max(vp,vt), sum min(vp,vt)]:
      sum|Vp-Vt| = col30 - col31."""
    total = np.float64(0.0)
    for p in partials_list:
        p = p.astype(np.float64)
        b = p[:, : 3 * N_BULK]
        total += 2.0 * b[:, 0::3].sum() - b[:, 1::3].sum() - b[:, 2::3].sum()
        total += p[:, 3 * N_BULK].sum() - p[:, 3 * N_BULK + 1].sum()
    return np.array(total / N_PIX, dtype=np.float32)


def kernel(pred: np.ndarray, target: np.ndarray) -> np.ndarray:
    from concourse.bass_utils import run_bass_kernel_spmd

    nc = _get_program()
    pred = np.ascontiguousarray(pred, dtype=np.float32).reshape(
        N_CORES, N_IMG, C, P, F
    )
    target = np.ascontiguousarray(target, dtype=np.float32).reshape(
        N_CORES, N_IMG, C, P, F
    )
    in_maps = [{"pred": pred[i], "target": target[i]} for i in range(N_CORES)]
    res = run_bass_kernel_spmd(nc, in_maps, list(range(N_CORES)))
    return _finish([r["partials"] for r in res.results])


# revision 12
# speedup vs baseline: 1.1109x; 1.1109x over previous
"""BrightnessLoss Trainium2 kernel (raw Bass, 8-core data parallel).

reference:
    V(x)   = max_c(clip(x, 0, 1))        over channel dim (RGB)
    result = mean(|V(pred) - V(target)|) over (N, H, W)

Identities used on device:
    clip(max(r,g,b),0,1) == max_c(clip(x,0,1))          (clip is monotone)
    W := relu(1 - relu(m)) == 1 - clip(m, 0, 1)
    |Vp - Vt| == |Wp - Wt|
    sum|Wp - Wt| == 2*sum max(Wp,Wt) - sum Wp - sum Wt

Work is cut into "units" (image chunks along the plane's free dim). Per unit:
    dma pred+targ [128, 3*w] f32  — even units on the SP HWDGE ring, odd
        units on the ACT ring, so one ring's inter-transfer bubble is
        covered by the other ring streaming the next unit
    DVE   m1 = max(R,G); u = (m1 max 0) max B   (fused relu, x2 sides)
    ACT   W = Relu(-u + 1), accum_out = sum(W)  (side sums come free)
    DVE   stt bypass,max: max(Wp,Wt), accum_out = sum
The last image's final chunks shrink (e.g. 1024,768,256) so the post-DMA
dependency chain of the very last unit is short. Partials are written out in
two DMAs (bulk early, last units at the end). Host combines in float64.
"""

import numpy as np

N_CORES = 8
N_IMG = 4  # 32 / 8
C = 3
P = 128
F = 2048  # 512*512 / 128
N_PIX = 32 * 512 * 512
N_CHUNKS = 2  # chunks per plane
TAIL_SPLIT = (768, 256)  # last image final-chunk split (sums to F/N_CHUNKS)


def _plan_units(n_img, f, n_chunks, tail_split):
    """Units: (img, col_offset, width). Last image's final chunk is split
    further per tail_split to shorten the end-of-kernel dependency chain."""
    fc = f // n_chunks
    units = []
    for img in range(n_img):
        offs = [(j * fc, fc) for j in range(n_chunks)]
        if img == n_img - 1 and tail_split:
            assert sum(tail_split) == fc
            off0 = offs[-1][0]
            offs = offs[:-1]
            o = off0
            for w in tail_split:
                offs.append((o, w))
                o += w
        for off, w in offs:
            units.append((img, off, w))
    return units, fc


def _build_program(n_img=N_IMG, f=F, n_chunks=N_CHUNKS, tail_split=TAIL_SPLIT):
    from contextlib import ExitStack

    import concourse.bass as bass
    import concourse.mybir as mybir

    fp32 = mybir.dt.float32
    Alu = mybir.AluOpType
    Act = mybir.ActivationFunctionType

    assert f % n_chunks == 0
    units, fc = _plan_units(n_img, f, n_chunks, tail_split)
    n_units = len(units)

    # detect_race_conditions=False: the raw-mode CoreSim race detector can't
    # see same-engine program-order (DVE m1 -> STT RAW); hardware engines
    # execute in order.
    # The class-level patch suppresses the framework barrier at the end of
    # Bass.__init__ (after the const-AP memsets): engines then enter the body
    # without a rendezvous and the first input DMA issues ~0.4us earlier.
    # The only preamble state the body reads is the const-1.0 bias tile
    # (ACT Relu bias); the gpsimd guard sem below restores that one edge.
    _cls_aeb = bass.Bass.all_engine_barrier
    bass.Bass.all_engine_barrier = lambda *a, **k: None
    try:
        nc = bass.Bass(
            "TRN2", target_bir_lowering=False, debug=False, detect_race_conditions=False
        )
    finally:
        bass.Bass.all_engine_barrier = _cls_aeb
    pred = nc.dram_tensor("pred", [n_img, C, P, f], fp32, kind="ExternalInput").ap()
    targ = nc.dram_tensor("target", [n_img, C, P, f], fp32, kind="ExternalInput").ap()
    out = nc.dram_tensor(
        "partials", [P, 3 * n_units], fp32, kind="ExternalOutput"
    ).ap()

    with ExitStack() as ctx:
        sb = lambda name, shape: ctx.enter_context(nc.sbuf_tensor(name, shape, fp32))
        sem = lambda name: ctx.enter_context(nc.semaphore(name))

        inb = [[sb(f"in{sl}{s}", [P, C * fc]) for s in range(2)] for sl in range(2)]
        ub = [[sb(f"u{sl}{s}", [P, fc]) for s in range(2)] for sl in range(2)]
        wb = [[sb(f"w{sl}{s}", [P, fc]) for s in range(2)] for sl in range(2)]
        m1 = sb("m1", [P, fc])
        scr = sb("stt_scratch", [P, fc])
        acc = sb("acc", [P, 3 * n_units])

        guard_buf = sb("guard_buf", [P, 1])

        inp_sem = [sem("inp0"), sem("inp1")]  # pred side, by slot parity
        int_sem = [sem("int0"), sem("int1")]  # targ side, by slot parity
        u_sem = sem("u")
        act_sem = sem("act")
        gp_sem = sem("gp")
        out_sem = sem("outd")
        cready = sem("cready")  # gpsimd: preamble const-AP memsets retired

        def dma_in(eng, side_idx, u):
            img, off, w = units[u]
            side = (pred, targ)[side_idx]
            s_sem = (inp_sem, int_sem)[side_idx]
            src = side[img, :, :, off : off + w].rearrange("c p f -> p c f")
            eng.dma_start(
                out=inb[u % 2][side_idx][:, : C * w].rearrange(
                    "p (c f) -> p c f", c=C
                ),
                in_=src,
            ).then_inc(s_sem[u % 2], 16)

        block = ctx.enter_context(nc.Block(no_gpsimd_drain=True))

        @block.gpsimd
        def _(gp):
            # Runs after the preamble const-AP memsets in gpsimd program
            # order; signals ACT that the const-1.0 bias tile is valid.
            gp.memset(guard_buf[:, :], 0.0).then_inc(cready, 1)

        @block.sync
        def _(sync):
            # even units ride the SP ring; odd units are issued from the ACT
            # stream (second HWDGE ring)
            for u in range(0, n_units, 2):
                if u >= 2:
                    # WAR inb[0][pred]: unit u-2's up STT (its last reader)
                    sync.wait_ge(u_sem, 2 * u - 3)
                dma_in(sync, 0, u)
                if u >= 2:
                    # WAR inb[0][targ]: unit u-2's ut STT (its last reader)
                    sync.wait_ge(u_sem, 2 * u - 2)
                dma_in(sync, 1, u)
            if n_units > 2:
                # bulk of partials early; only the last 2 units' cols remain
                sync.wait_ge(gp_sem, n_units - 2)
                sync.dma_start(
                    out=out[:, : 3 * (n_units - 2)],
                    in_=acc[:, : 3 * (n_units - 2)],
                ).then_inc(out_sem, 16)
            sync.wait_ge(gp_sem, n_units)
            # No out_sem wait after the final write: the block-exit drain
            # fences the HWDGE ring before NEFF completion.
            sync.dma_start(
                out=out[:, 3 * max(0, n_units - 2) :],
                in_=acc[:, 3 * max(0, n_units - 2) :],
            ).then_inc(out_sem, 16)

        @block.vector
        def _(vector):
            def accum(u):
                # max(Wp, Wt) elementwise, accum_out = per-partition sum
                w = units[u][2]
                vector.wait_ge(act_sem, 2 * (u + 1))
                vector.scalar_tensor_tensor(
                    scr[:, :w],
                    wb[u % 2][0][:, :w],
                    0.0,
                    wb[u % 2][1][:, :w],
                    op0=Alu.bypass,
                    op1=Alu.max,
                    accum_out=acc[:, 3 * u : 3 * u + 1],
                ).then_inc(gp_sem, 1)

            for u in range(n_units):
                w = units[u][2]
                for s in range(2):
                    vector.wait_ge((inp_sem, int_sem)[s][u % 2], 16 * (u // 2 + 1))
                    t = inb[u % 2][s]
                    vector.tensor_max(m1[:, :w], t[:, 0:w], t[:, w : 2 * w])
                    if u >= 2:
                        # WAR on ub[u%2][s]: ACT's W of unit u-2 (its reader)
                        vector.wait_ge(act_sem, 2 * (u - 1))
                    vector.scalar_tensor_tensor(
                        ub[u % 2][s][:, :w],
                        m1[:, :w],
                        0.0,
                        t[:, 2 * w : 3 * w],
                        op0=Alu.max,
                        op1=Alu.max,
                    ).then_inc(u_sem, 1)
                if u > 0:
                    accum(u - 1)
            accum(n_units - 1)

        @block.scalar
        def _(scalar):
            # odd units' input DMAs ride the ACT HWDGE ring. Unit 1 goes up
            # front (fresh slot, no WAR); unit n+2 is placed right after
            # W_{n,1}, whose u_sem wait (>= 2n+2 = 2(n+2)-2) already covers
            # both WAR conditions for slot (n+2)%2.
            if n_units > 1:
                dma_in(scalar, 0, 1)
                dma_in(scalar, 1, 1)
            scalar.wait_ge(cready, 1)  # const-1.0 bias tile valid
            for n in range(n_units):
                w = units[n][2]
                for s in range(2):
                    scalar.wait_ge(u_sem, 2 * n + s + 1)
                    if n >= 2:
                        # WAR on wb[n%2][s]: accum of unit n-2 (its reader)
                        scalar.wait_ge(gp_sem, n - 1)
                    scalar.activation(
                        wb[n % 2][s][:, :w],
                        ub[n % 2][s][:, :w],
                        Act.Relu,
                        bias=1.0,
                        scale=-1.0,
                        accum_out=acc[:, 3 * n + 1 + s : 3 * n + 2 + s],
                    ).then_inc(act_sem, 1)
                if n + 2 < n_units and (n + 2) % 2 == 1:
                    dma_in(scalar, 0, n + 2)
                    dma_in(scalar, 1, n + 2)

        # Skip the Block-exit all-engine barrier (~4.3us): every cross-engine
        # dependency is semaphore-gated and the per-engine exit drains
        # (no_gpsimd_drain path) still fence the DMA rings, so engines may
        # halt independently — NEFF completion waits for all engines anyway.
        nc.all_engine_barrier = lambda *a, **k: None

    del nc.all_engine_barrier  # restore class method
    return nc


_program = None


def _get_program():
    global _program
    if _program is None:
        _program = _build_program()
    return _program


def _finish(partials_list):
    """partials_list: per-core [P, 3*n_units] f32 with cols per unit:
    [sum max(Wp,Wt), sum Wp, sum Wt].
    sum|Vp-Vt| = 2*sum(max) - sum(Wp) - sum(Wt)."""
    total = np.float64(0.0)
    for p in partials_list:
        p = p.astype(np.float64)
        total += 2.0 * p[:, 0::3].sum() - p[:, 1::3].sum() - p[:, 2::3].sum()
    return np.array(total / N_PIX, dtype=np.float32)


def kernel(pred: np.ndarray, target: np.ndarray) -> np.ndarray:
    from concourse.bass_utils import run_bass_kernel_spmd

    nc = _get_program()
    pred = np.ascontiguousarray(pred, dtype=np.float32).reshape(
        N_CORES, N_IMG, C, P, F
    )
    target = np.ascontiguousarray(target, dtype=np.float32).reshape(
        N_CORES, N_IMG, C, P, F
    )
    in_maps = [{"pred": pred[i], "target": target[i]} for i in range(N_CORES)]
    res = run_bass_kernel_spmd(nc, in_maps, list(range(N_CORES)))
    return _finish([r["partials"] for r in res.results])

